# revision 30
# baseline (speedup 1.0000x reference)
"""Trainium2 Bass kernel for CausalPriorityAttention.

Data-parallel over the batch dim: core b computes batch b (B=8, 8 cores).

v7: power/schedule revision of the v4 fp16 pipeline, tuned against the
real HW duty-cycle governor (HAM type-1 throttle: sustained load drops
the core to 4/8 duty, so total engine-seconds are the binding budget).
The PE work is already at its output floor (~193k cycles), so v7 cuts
everything that burns power or convoys around the PE:
  - softmax 1/r = exp(-ln r) runs on the mostly-idle ACT engine (the
    6-cycle-per-element DVE reciprocal was 53us of DVE critical path);
    a Bacc subclass keeps the activation-table pass from thrashing
    1.3us table loads between the Ln and Exp sets (2 loads/rep);
  - each hp-loop iteration interleaves PV+normalization of pair hp with
    the scores/exps/multiplies of pair hp+1 in 4 slices, so no in-order
    engine queue convoys behind another engine's latency;
  - both subs' E-multiplies fuse into one [P,2,N] fp16-2x DVE
    instruction via a stride-0 broadcast view of E;
  - the out-proj bias rank-1 matmuls are gone (bias rides the DVE
    fp16 copy-out against a partition-replicated bias tile);
  - W^T/Wo^T/biases load once per NEFF; only x^T + graph_bias stream
    per rep; y is written back fp16 (upcast on host);
  - the next rep's input DMAs issue at hp==2; its tanh/E pass, QKV
    blocks and pair-0 scores all run in the ph3 tail (a norm-free ACT
    window, away from Ln tables), ordered blocks-p0 -> scores-p0 ->
    remaining blocks so the score exps start draining on ACT while the
    PE streams the projection matmuls: neither engine idles across the
    rep boundary.

Per-core dataflow (512-wide matmuls, fp16 operands, f32 psum):
  phase 1: qkT = W_qk @ x^T  (Q^T,K^T in [feat, seq] layout)
           V   = x @ W_v^T   (natural [seq, feat] layout, +ones col)
           E   = exp(5a*tanh(x/2) + 5a) = exp(10a*sigmoid(x))
  phase 2 (per head pair, row-group-paired K=64 score matmuls):
           sT[k,q] = K @ Q'^T            (transposed scores -> PSUM)
           probs   = exp(10a*sT - 5) * E (ACT exp + DVE fp16-2x mult)
           pv[65,q] = [V_h | 1]^T @ probs  (out^T rows + rowsums)
           attnT = pv[0:64] * exp(-ln pv[64])  (ACT recip chain,
           Pool partition_broadcast, DVE multiply)
  phase 3: y = attnT^T @ Wo^T, bias added on the fp16 copy-out
Q' is prescaled by 1/(8*10a) so exp's scale=10a restores QK/8; the
reference bias's constant -5a term drops out of softmax, and the -5 shift
(which also cancels in normalization) keeps exp products in fp16 range.
The transposed-score layout means graph_bias is consumed untransposed and
probs come out pre-transposed for the PV matmul: zero on-chip transposes.
"""

import sys

for _p in ("/opt/trn_rl_repo",):
    if _p not in sys.path:
        sys.path.append(_p)

import numpy as np

import concourse.bacc as bacc
import concourse.bass as bass
import concourse.mybir as mybir
import concourse.tile as tile
from concourse.bass_utils import run_bass_kernel_spmd

B, N, D = 8, 1024, 512
H, HD = 8, 64
P = 128
NT = N // P          # 8 seq tiles
KT = D // P          # 4 contraction tiles over D
FT_QK = 2 * D // P   # 8 feature tiles over [Q;K]
QC = N // 512        # 2 q-chunks of 512
F32 = mybir.dt.float32
F16 = mybir.dt.float16

_CACHE = {}


class _Bacc(bacc.Bacc):
    """Bacc whose activation-table-load pass never picks the exp-less
    `natural_log` set: the stock pass resolves Ln by first match, which
    lands on `natural_log` and then thrashes a 1.3us table load around
    every Ln<->Exp alternation (the softmax 1/r = exp(-ln r) chains).
    Presenting that set as empty (positions, and therefore the emitted
    act_func_set_ids, are unchanged) makes Ln resolve to
    `natural_log_exp_and_others`, which also holds Exp: 2 loads per rep."""

    def insert_act_table_loads(self):
        import bass_rust as _br
        from concourse.hw_specs import get_activation_tables

        if not any(
            isinstance(i, mybir.InstActivation)
            for b in self.main_func.blocks
            for i in b.instructions
        ):
            return
        tabs = [
            (n, (set() if n == "natural_log" else s))
            for n, s in get_activation_tables(self.m.arch).items()
        ]
        _br.insert_act_table_loads(self, tabs)


def build_nc(ten_a: float, reps: int = 1):
    nc = _Bacc("TRN2")
    xT = nc.dram_tensor("xT", [D, N], F16, kind="ExternalInput")
    wT = nc.dram_tensor("wT", [D, 3 * D], F16, kind="ExternalInput")
    gb = nc.dram_tensor("gb", [N, N], F16, kind="ExternalInput")
    woT = nc.dram_tensor("woT", [D, D], F16, kind="ExternalInput")
    qkb = nc.dram_tensor("qkb", [P, FT_QK], F32, kind="ExternalInput")
    vb = nc.dram_tensor("vb", [D], F16, kind="ExternalInput")
    bo = nc.dram_tensor("bo", [1, D], F16, kind="ExternalInput")
    y = nc.dram_tensor("y", [N, D], F16, kind="ExternalOutput")

    sQ = 1.0 / (8.0 * ten_a)

    with tile.TileContext(nc) as tc:
        with tc.tile_pool(name="const", bufs=1) as const_pool, \
             tc.tile_pool(name="persist", bufs=1) as persist, \
             tc.tile_pool(name="ph1", bufs=1) as ph1, \
             tc.tile_pool(name="ph2", bufs=1) as ph2, \
             tc.tile_pool(name="small", bufs=2) as small, \
             tc.tile_pool(name="ps_s", bufs=3, space="PSUM") as ps_s, \
             tc.tile_pool(name="ps_acc", bufs=2, space="PSUM") as ps_acc:
            qkb_sb = const_pool.tile([P, FT_QK], F32)
            nc.sync.dma_start(out=qkb_sb, in_=qkb[:, :])
            vb_sb = const_pool.tile([P, D], F16)
            nc.sync.dma_start(
                out=vb_sb,
                in_=bass.AP(tensor=vb.ap().tensor, offset=0, ap=[[0, P], [1, D]]),
            )
            # out-proj bias replicated across partitions for the copy-out add
            bo_bc = const_pool.tile([P, D], F16)
            nc.sync.dma_start(
                out=bo_bc,
                in_=bass.AP(tensor=bo.ap().tensor, offset=0, ap=[[0, P], [1, D]]),
            )
            neg5 = const_pool.tile([P, 1], F32)
            nc.vector.memset(neg5, -5.0)
            p5a = const_pool.tile([P, 1], F32)
            nc.vector.memset(p5a, ten_a / 2.0)

            # weights persist across reps: one DMA per NEFF
            wT_sb = persist.tile([P, KT, 3 * D], F16)
            nc.sync.dma_start(
                out=wT_sb, in_=wT[:, :].rearrange("(t p) n -> p t n", p=P)
            )
            woT_sb = persist.tile([P, KT, D], F16)
            nc.gpsimd.dma_start(
                out=woT_sb, in_=woT[:, :].rearrange("(t p) n -> p t n", p=P)
            )

            qkT_sb = persist.tile([P, FT_QK, N], F16)
            v_sb = persist.tile([P, NT, H, HD + 1], F16)
            nc.gpsimd.memset(v_sb[:, :, :, HD : HD + 1], 1.0)
            # double-buffered per rep so the next rep's E pass can run in
            # this rep's tail while this rep's multiplies still read E
            e_tiles = [persist.tile([P, NT, N], F16, name="e0"),
                       persist.tile([P, NT, N], F16, name="e1")]
            attnT_sb = persist.tile([P, KT, N], F16)

            # ---------- emission helpers ----------

            def emit_in_dmas():
                """xT + graph_bias DMAs for one rep. Returns (xT_sb, gts)."""
                xT_sb = ph1.tile([P, KT, N], F16, tag="xT", bufs=2,
                                 name="xT_sb")
                for k in range(KT):
                    nc.sync.dma_start(
                        out=xT_sb[:, k, :], in_=xT[k * P : (k + 1) * P, :]
                    )
                gts = []
                for k2 in range(NT // 2):
                    gt = ph1.tile([P, 2, N], F16, tag="gbt2", bufs=2,
                                  name="gbt2")
                    for j in range(2):
                        kt = 2 * k2 + j
                        nc.gpsimd.dma_start(
                            out=gt[:, j, :], in_=gb[kt * P : (kt + 1) * P, :]
                        )
                    gts.append(gt)
                return xT_sb, gts

            def emit_epass(gts, eb):
                """tanh + exp ACT passes over graph_bias. Emitted only in
                norm-free windows (rep tail): Tanh lives in a different ACT
                table set than Ln, so this must not interleave with the
                softmax-normalization chains."""
                sgs = []
                for k2 in range(NT // 2):
                    sg = ph1.tile([P, 2, N], F16, tag="sig2", bufs=1,
                                  name="sig2")
                    nc.scalar.activation(
                        out=sg, in_=gts[k2],
                        func=mybir.ActivationFunctionType.Tanh,
                        scale=0.5,
                    )
                    sgs.append(sg)
                for k2 in range(NT // 2):
                    nc.scalar.activation(
                        out=eb[:, 2 * k2 : 2 * k2 + 2, :],
                        in_=sgs[k2],
                        func=mybir.ActivationFunctionType.Exp,
                        scale=ten_a / 2.0,
                        bias=p5a,
                    )

            def emit_blocks(groups, xT_sb):
                """QKV projection groups, k outermost within blocks of 2 psum
                groups so k=0..2 matmuls run as each xT DMA chunk lands."""
                for b0 in range(0, len(groups), 2):
                    blk = groups[b0 : b0 + 2]
                    tiles = [ps_acc.tile([P, 512], F32, tag="acc", name="ps")
                             for _ in blk]
                    for k in range(KT):
                        for g, t in zip(blk, tiles):
                            if g[0] == "qk":
                                _, ft, qc = g
                                nc.tensor.matmul(
                                    t,
                                    lhsT=wT_sb[:, k, ft * P : (ft + 1) * P],
                                    rhs=xT_sb[:, k, qc * 512 : (qc + 1) * 512],
                                    start=(k == 0),
                                    stop=(k == KT - 1),
                                )
                            else:
                                _, st, _ = g
                                nc.tensor.matmul(
                                    t,
                                    lhsT=xT_sb[:, k, st * P : (st + 1) * P],
                                    rhs=wT_sb[:, k, 2 * D : 3 * D],
                                    start=(k == 0),
                                    stop=(k == KT - 1),
                                )
                    for g, t in zip(blk, tiles):
                        if g[0] == "qk":
                            _, ft, qc = g
                            # copy out with per-partition bias + Q prescale
                            nc.vector.tensor_scalar(
                                out=qkT_sb[:, ft, qc * 512 : (qc + 1) * 512],
                                in0=t,
                                scalar1=qkb_sb[:, ft : ft + 1],
                                scalar2=(sQ if ft < FT_QK // 2 else 1.0),
                                op0=mybir.AluOpType.add,
                                op1=mybir.AluOpType.mult,
                            )
                        else:
                            _, st, _ = g
                            nc.vector.tensor_tensor(
                                out=v_sb[:, st, :, 0:HD],
                                in0=t.rearrange("p (h d) -> p h d", h=H),
                                in1=vb_sb.rearrange("p (h d) -> p h d", h=H),
                                op=mybir.AluOpType.add,
                            )

            def g_pair(hp):
                return [("qk", ft, qc) for ft in (hp, 4 + hp)
                        for qc in range(QC)]

            G_V = [("v", st, 0) for st in range(NT)]

            def emit_scores_exps(hp, kts):
                """Score matmuls + ACT exps for head pair hp over the given
                kt chunk. Both subs' exps write halves of one [P, 2, N] tile
                so the downstream DVE multiply covers a whole kt in one
                instruction."""
                es_tiles = []
                for kt in kts:
                    sT2 = [
                        ps_s.tile([P, N], F32, tag="sT", name="sTa"),
                        ps_s.tile([P, N], F32, tag="sT", name="sTb"),
                    ]
                    for qc in range(QC):
                        for sub in range(2):
                            qp = 64 * sub
                            nc.tensor.matmul(
                                sT2[sub][:, qc * 512 : (qc + 1) * 512],
                                lhsT=qkT_sb[
                                    qp : qp + HD,
                                    FT_QK // 2 + hp,
                                    kt * P : (kt + 1) * P,
                                ],
                                rhs=qkT_sb[
                                    qp : qp + HD, hp, qc * 512 : (qc + 1) * 512
                                ],
                                start=True,
                                stop=True,
                            )
                    es2 = ph2.tile([P, 2, N], F16, tag="es", bufs=4,
                                   name="es2")
                    for sub in range(2):
                        # -5 keeps exp(s)*exp(bias) products in fp16 range;
                        # the shift cancels in normalization
                        nc.scalar.activation(
                            out=es2[:, sub, :],
                            in_=sT2[sub],
                            func=mybir.ActivationFunctionType.Exp,
                            scale=ten_a,
                            bias=neg5,
                        )
                    es_tiles.append((kt, es2))
                return es_tiles

            def new_expT():
                return ph2.tile([P, NT, 2, N], F16, tag="exp", bufs=2,
                                name="expT")

            def emit_mults(es_tiles, expT, eb):
                for kt, es2 in es_tiles:
                    ebs = eb[:, kt, :]
                    # stride-0 broadcast of E over the sub dim: one fp16-2x
                    # DVE instruction covers both subs of a kt
                    eb_bc = bass.AP(
                        tensor=ebs.tensor, offset=ebs.offset,
                        ap=[ebs.ap[0], [0, 2], [1, N]],
                    )
                    nc.vector.tensor_tensor(
                        out=expT[:, kt, :, :],
                        in0=es2,
                        in1=eb_bc,
                        op=mybir.AluOpType.mult,
                    )

            def emit_pv_group(hp, expT, qc, sub):
                """One (qc, sub) PV accumulation + its 1/r = exp(-ln r)
                recip chain (ACT) + Pool broadcast. Returns the pending
                attnT-multiply args."""
                h = 2 * hp + sub
                acc = ps_acc.tile([P, 512], F32, tag="acc", name="acc")
                pv = acc[0 : HD + 1, :]
                for kt in range(NT):
                    nc.tensor.matmul(
                        pv,
                        lhsT=v_sb[:, kt, h, :],
                        rhs=expT[:, kt, sub, qc * 512 : (qc + 1) * 512],
                        start=(kt == 0),
                        stop=(kt == NT - 1),
                    )
                lr = small.tile([1, 512], F32, tag="lr", name="lr")
                nc.scalar.activation(
                    out=lr, in_=pv[HD : HD + 1, :],
                    func=mybir.ActivationFunctionType.Ln,
                )
                recip = small.tile([1, 512], F32, tag="recip", name="recip")
                nc.scalar.activation(
                    out=recip, in_=lr,
                    func=mybir.ActivationFunctionType.Exp,
                    scale=-1.0,
                )
                bc = small.tile([HD, 512], F32, tag="bc", name="bc")
                nc.gpsimd.partition_broadcast(bc, recip)
                return (qc, sub, pv, bc)

            def emit_attn_mult(hp, qc, sub, pv, bc):
                qp = 64 * sub
                nc.vector.tensor_tensor(
                    out=attnT_sb[
                        qp : qp + HD, hp, qc * 512 : (qc + 1) * 512
                    ],
                    in0=pv[0:HD, :],
                    in1=bc,
                    op=mybir.AluOpType.mult,
                )

            def emit_iteration(hp, expT_cur, expT_n, eb, dma_hook=None):
                """One hp-loop iteration: PV+norm of pair hp interleaved in
                4 slices with the scores+exps+multiplies of pair hp+1 (when
                expT_n is given) so no engine queue convoys behind another;
                for the last pair (expT_n None) ph3(qc) is emitted as soon
                as qc's attnT tiles land."""
                groups = [(qc, sub) for qc in range(QC) for sub in range(2)]
                pend = None
                for j, (qc, sub) in enumerate(groups):
                    es = None
                    if expT_n is not None:
                        es = emit_scores_exps(hp + 1, [2 * j, 2 * j + 1])
                    g = emit_pv_group(hp, expT_cur, qc, sub)
                    if es is not None:
                        emit_mults(es, expT_n, eb)
                    if pend is not None:
                        emit_attn_mult(hp, *pend)
                        if expT_n is None and pend[0] == 0 and pend[1] == 1:
                            emit_ph3(0)
                    pend = g
                    if j == 1 and dma_hook is not None:
                        dma_hook()
                emit_attn_mult(hp, *pend)
                if expT_n is None:
                    emit_ph3(1)

            def emit_ph3(qc):
                for st in range(qc * NT // 2, (qc + 1) * NT // 2):
                    yp = ps_acc.tile([P, D], F32, tag="acc", name="yp")
                    for ft in range(KT):
                        nc.tensor.matmul(
                            yp,
                            lhsT=attnT_sb[:, ft, st * P : (st + 1) * P],
                            rhs=woT_sb[:, ft, :],
                            start=(ft == 0),
                            stop=(ft == KT - 1),
                        )
                    ysb = ph2.tile([P, D], F16, tag="ysb", bufs=2, name="ysb")
                    # fused bias add on the fp16 copy-out
                    nc.vector.tensor_tensor(
                        out=ysb, in0=yp, in1=bo_bc,
                        op=mybir.AluOpType.add,
                    )
                    nc.gpsimd.dma_start(
                        out=y[st * P : (st + 1) * P, :], in_=ysb
                    )

            # ---------- rep loop ----------
            # Emission order IS the per-engine program order. Per rep:
            # phase 1 with pair-0's blocks+scores leading into the hp loop;
            # each iteration interleaves PV+norm of pair hp with scores/
            # exps/multiplies of pair hp+1 in 4 slices. The next rep's
            # input DMAs issue at hp==2; its tanh/E pass runs in the ph3
            # tail (a norm-free ACT window: Tanh and Ln live in different
            # ACT table sets, so keeping them apart avoids table thrash).
            G_REST = g_pair(1) + g_pair(2) + g_pair(3) + G_V

            es0 = expT0 = xT_cur = gts_cur = None
            for r in range(reps):
                eb = e_tiles[r % 2]
                if r == 0:
                    # first rep: everything up front (no tail to hide in)
                    xT_cur, gts_cur = emit_in_dmas()
                    emit_epass(gts_cur, eb)
                    emit_blocks(g_pair(0), xT_cur)
                    expT0 = new_expT()
                    es0 = emit_scores_exps(0, list(range(NT)))
                    emit_blocks(G_REST, xT_cur)

                emit_mults(es0, expT0, eb)
                expT = {0: expT0}
                for hp in range(H // 2):
                    nxt = hp + 1
                    hook = None
                    if hp == 2 and r + 1 < reps:
                        def hook():
                            nonlocal xT_cur, gts_cur
                            xT_cur, gts_cur = emit_in_dmas()
                    if nxt < H // 2:
                        expT[nxt] = new_expT()
                        emit_iteration(hp, expT[hp], expT[nxt], eb,
                                       dma_hook=hook)
                    else:
                        emit_iteration(hp, expT[hp], None, eb)
                if r + 1 < reps:
                    # next rep's FULL prologue in this rep's ph3 tail: the
                    # E pass + all QKV blocks keep PE and ACT both fed here
                    # (previously the G_REST blocks ran at rep start with
                    # ACT idle for ~15us)
                    eb2 = e_tiles[(r + 1) % 2]
                    emit_epass(gts_cur, eb2)
                    emit_blocks(g_pair(0), xT_cur)
                    # pair-0 scores BEFORE the remaining blocks: their ACT
                    # exps start draining ~6us into the tail instead of
                    # stalling ACT ~19us behind 40 projection matmuls
                    expT0 = new_expT()
                    es0 = emit_scores_exps(0, list(range(NT)))
                    emit_blocks(G_REST, xT_cur)
    nc.finalize()
    return nc


def kernel(x, graph_bias, in_proj_w, in_proj_b, out_proj_w, out_proj_b,
           bias_strength):
    x = np.asarray(x, dtype=np.float32)
    graph_bias = np.asarray(graph_bias, dtype=np.float32)
    in_proj_w = np.asarray(in_proj_w, dtype=np.float32)
    in_proj_b = np.asarray(in_proj_b, dtype=np.float32)
    out_proj_w = np.asarray(out_proj_w, dtype=np.float32)
    out_proj_b = np.asarray(out_proj_b, dtype=np.float32)
    alpha = 1.0 / (1.0 + np.exp(-float(np.asarray(bias_strength))))
    ten_a = 10.0 * alpha

    key = round(ten_a, 9)
    if key not in _CACHE:
        _CACHE[key] = build_nc(ten_a)
    nc = _CACHE[key]

    wT = np.ascontiguousarray(in_proj_w.T).astype(np.float16)  # [512, 1536]
    woT = np.ascontiguousarray(out_proj_w.T).astype(np.float16)
    qkb = np.ascontiguousarray(
        in_proj_b[: 2 * D].reshape(FT_QK, P).T      # [128, 8]
    )
    vb = in_proj_b[2 * D :].astype(np.float16)
    bo = out_proj_b.astype(np.float16)
    gb16 = graph_bias.astype(np.float16)

    in_maps = []
    for b in range(B):
        in_maps.append({
            "xT": np.ascontiguousarray(x[b].T).astype(np.float16),
            "wT": wT,
            "gb": np.ascontiguousarray(gb16[b]),
            "woT": woT,
            "qkb": qkb,
            "vb": vb,
            "bo": bo.reshape(1, D),
        })

    global _saved_in_maps
    _saved_in_maps = in_maps
    res = run_bass_kernel_spmd(nc, in_maps, core_ids=list(range(B)))
    out = np.stack([res.results[b]["y"] for b in range(B)], axis=0)
    return out.astype(np.float32)
